# revision 62
# baseline (speedup 1.0000x reference)
"""Trainium2 Bass kernel for nn_AttentionBlock (B=4, L=2048, C=512, H=8, Dk=64).

Sharding (8 cores): data-parallel over B (4) x tensor-parallel over heads (2
groups of 4). Core c handles batch c//2, head group c%2. Each core computes
  y_c = attention(x_b)[:, local_heads] @ W_out[local_rows]        [2048, 512]
and the host combines: out[b] = y[2b] + y[2b+1] + b_out + x[b].

Device kernel (per core); matmul operands fp16, fp32 PSUM accumulation:
  - qT/kT per head in [Dk, L] layout straight out of the projection
    (lhsT=W_in chunk, rhs=xT chunk) -- no transposes anywhere. Each head
    owns a full [128, L] tile whose complementary 64 rows are kept zero
    (mask fused into the projection copy), so every ST matmul is a
    uniform K=128 / (128,128)-tile op: mixing 64- and 128-row weight
    tiles costs a ~90ns PE array-reconfig stall per switch.
  - v in natural [L, Dk] layout, augmented with a ones column (plus zero
    padding to 128, written once at startup) so the O^T = V^T P^T matmul
    also produces the softmax denominators for free.
  - scores S^T [keys, queries]; causal structure skips upper-triangle
    tiles and narrows diagonal-straddling tiles; both diagonal 128x128
    blocks of a straddle pair get one batched 0/1 triangle multiply
    (3-dim AP, 640-col stride) after exp.
  - exp batched over key-tile pairs (one 2-bank PSUM tile), split per
    half on straddle pairs so only written PSUM is read; the
    ST->exp->OT chain is software-pipelined 2 pairs deep so the OT
    matmul never waits on ACT latency. ACT runs exp ONLY -- any copy
    routed through it queues ahead of exps and stalls the PE.
  - softmax normalization off the PE queue: DVE copy (custom-DVE ops
    must not read PSUM directly on HW) + fast reciprocal, GpSimd
    partition-broadcast, DVE multiply writing into a head-PAIR packed
    layout (head 2p in partitions 0:64, 2p+1 in 64:128) so the
    out-projection runs K=128 matmuls (2 per row tile, not 4).
  - emission order keeps the (strictly in-order) PE queue dense: a
    warmup burst covers the input-DMA wait + HAM clock ramp;
    projection/out-projection units are interleaved between attention
    pairs with a block-level budget matched to the ACT-vs-PE balance
    (the final block gets all deferred out-projections); input DMA is
    spread across three engine queues with the startup-critical slices
    (xT cols 0:512, unit-major q/k weights) first.
fp16 operands keep absmax-relative error vs the fp32 reference at ~4e-4
(8x tighter than bf16) at identical PE throughput; y returned as fp16.
"""

import sys

sys.path.insert(0, "/opt/trn_rl_repo")

import numpy as np

import concourse.bacc as bacc
import concourse.bass as bass
import concourse.mybir as mybir
import concourse.tile as tile
from concourse.bass_utils import run_bass_kernel_spmd

# ---------------------------------------------------------------- constants
B, L, C = 4, 2048, 512
H, DK = 8, 64
HPC = 4  # heads per core
SCALE = DK**-0.5
N_CORES = 8
KC = C // 128  # 4 contraction chunks
LT = L // 128  # 16 row tiles
QB = L // 512  # 4 query blocks of 512

F32 = mybir.dt.float32
BF16 = mybir.dt.bfloat16
F16 = mybir.dt.float16

# matmul operand dtype: "fp16" (fast, accurate) / "bf16" / "fp32" (exact)
MM_MODE = "fp16"

# test hooks (grading path leaves these alone)
TRACE = False
LAST_RESULT = None

_CACHE = {}


def _np_mm_dtype():
    if MM_MODE == "bf16":
        import ml_dtypes

        return ml_dtypes.bfloat16
    if MM_MODE == "fp16":
        return np.float16
    return np.float32


def _mm_dt():
    return {"bf16": BF16, "fp16": F16, "fp32": F32}[MM_MODE]


def _build(mm_mode):
    mm = {"bf16": BF16, "fp16": F16, "fp32": F32}[mm_mode]
    nc = bacc.Bacc(None)

    xT = nc.declare_dram_parameter("xT", [C, L], mm, isOutput=False)
    # unit-major (and pre-transposed to partition-major rows) so each
    # unit's weights arrive in one contiguous DMA, letting the prelude
    # start as soon as the first unit's slice lands
    w_in_qk = nc.declare_dram_parameter("w_in_qk", [HPC, 128, KC, 128], mm, isOutput=False)
    w_in_v = nc.declare_dram_parameter("w_in_v", [C, HPC, DK], mm, isOutput=False)
    # cols 0..7: q/k biases per (unit, half); col 8: low-half row mask
    # (partitions 0:64), col 9: high-half row mask (64:128)
    qkb = nc.declare_dram_parameter("qkb", [128, 10], F32, isOutput=False)
    vb = nc.declare_dram_parameter("vb", [HPC, DK + 1], F32, isOutput=False)
    w_out = nc.declare_dram_parameter("w_out", [128, 2, C], mm, isOutput=False)
    tri = nc.declare_dram_parameter("tri", [128, 128], mm, isOutput=False)
    y = nc.declare_dram_parameter("y", [L, C], F16, isOutput=True)

    with tile.TileContext(nc) as tc:
        with (
            tc.tile_pool(name="persist", bufs=1) as per,
            tc.tile_pool(name="work", bufs=2) as work,
            tc.tile_pool(name="psum", bufs=1, space="PSUM") as psum,
        ):
            # ---------------- loads
            xT_sb = [per.tile([128, L], mm, tag=f"xT{i}", name=f"xT{i}") for i in range(KC)]
            w_qk_sb = [per.tile([128, KC, 128], mm, tag=f"wq{u}", name=f"wq{u}") for u in range(HPC)]
            w_v_sb = [per.tile([128, HPC, DK], mm, tag=f"wv{i}", name=f"wv{i}") for i in range(KC)]
            w_out_sb = per.tile([128, 2, C], mm, tag="wo")
            tri_sb = per.tile([128, 128], mm, tag="tri")
            qkb_sb = per.tile([128, 10], F32, tag="qkb")
            vb_sb = per.tile([128, HPC, DK + 1], F32, tag="vb")

            # PE warmup: dependency-free dummy matmuls fill the input-DMA
            # wait and hold the HAM clock-gate warm before real work starts
            # (otherwise warm/cold entry is start-phase luck, ~+30us).
            warm = per.tile([128, 512], mm, tag="warm")
            nc.vector.memset(warm, 0.0)
            wps = psum.tile([128, 512], F32, tag="ot", bufs=2, name="warmps")
            for _ in range(10):
                nc.tensor.matmul(
                    wps, lhsT=warm[:, 0:128], rhs=warm, start=True, stop=True
                )

            # Input loads: DMA issue is ~0.6us per dma_start per engine
            # queue and each queue sustains ~100 GB/s, so spread the load
            # across four engine queues. The first projection unit needs
            # cols 0:512 of ALL four xT chunks plus the q/k weights (the
            # startup critical path), so those go first on their queues;
            # v weights ride the otherwise-idle vector queue.
            xT_t = xT.rearrange("(c p) l -> c p l", p=128)
            w_v_t = w_in_v.rearrange("(c p) h d -> c p h d", p=128)
            for i in range(KC):
                eng = nc.sync if i < 2 else nc.scalar
                eng.dma_start(out=xT_sb[i][:, 0:512], in_=xT_t[i][:, 0:512])
                nc.gpsimd.dma_start(out=w_qk_sb[i], in_=w_in_qk[i])
            for i in range(KC):
                nc.scalar.dma_start(out=w_v_sb[i], in_=w_v_t[i])
            nc.sync.dma_start(out=qkb_sb, in_=qkb[:, :])
            vb_ap = vb[:, :]
            vb_bcast = bass.AP(
                tensor=vb_ap.tensor, offset=vb_ap.offset, ap=[[0, 128], *vb_ap.ap]
            )
            nc.sync.dma_start(out=vb_sb, in_=vb_bcast)
            nc.sync.dma_start(out=tri_sb, in_=tri[:, :])
            for i in range(KC):
                eng = nc.sync if i < 2 else nc.scalar
                eng.dma_start(out=xT_sb[i][:, 512:L], in_=xT_t[i][:, 512:L])
            nc.scalar.dma_start(out=w_out_sb, in_=w_out[:, :, :])

            # ---------------- fused pipeline ----------------
            # Attention per key-tile pair: ST matmuls -> ACT exp -> OT
            # matmuls, software-pipelined two pairs deep; projection and
            # out-projection matmuls are fed into the PE queue one unit at
            # a time between pairs so the PE stays dense while ACT works.
            # heads 0,1 keep q/k in partitions 64:128; heads 2,3 in 0:64 --
            # one M=128 projection matmul serves two heads (host packs W_in
            # columns accordingly). Each head owns a full [128, L] tile whose
            # complementary 64 rows are KEPT ZERO (the projection copy
            # multiplies by a per-partition row mask), so every ST matmul is
            # a full K=128 / (128,128)-tile op -- mixing 64-row and 128-row
            # weight tiles on the PE costs an array-reconfig stall per
            # switch.
            qT_sb = [per.tile([128, L], mm, tag=f"qT{h}", name=f"qT{h}") for h in range(HPC)]
            kT_sb = [per.tile([128, L], mm, tag=f"kT{h}", name=f"kT{h}") for h in range(HPC)]

            # (unit, psum-half) -> (role tiles, head, tile row base)
            UNIT_DST = {
                (0, 0): (qT_sb, 2, 0), (0, 1): (qT_sb, 0, 64),
                (1, 0): (kT_sb, 2, 0), (1, 1): (kT_sb, 0, 64),
                (2, 0): (qT_sb, 3, 0), (2, 1): (qT_sb, 1, 64),
                (3, 0): (kT_sb, 3, 0), (3, 1): (kT_sb, 1, 64),
            }
            v_sb = [per.tile([128, HPC, 128], mm, tag=f"v{lt}", name=f"v{lt}") for lt in range(LT)]
            # constant regions of the v tiles (zero pad for fast weight
            # load + the ones column that produces softmax denominators):
            # written once here, in the input-DMA wait window, instead of
            # per v-unit in steady state
            for lt in range(LT):
                nc.vector.memset(v_sb[lt][:, :, DK:128], 0.0)
                nc.vector.memset(v_sb[lt][:, :, DK : DK + 1], 1.0)
            # attention output, head-PAIR packed: pair p holds head 2p in
            # partitions 0:64 and head 2p+1 in 64:128 -> out-projection
            # contracts K=128 (two heads per matmul).
            otp_sb = [per.tile([128, L], mm, tag=f"otp{p}", name=f"otp{p}") for p in range(2)]

            def emit_qk_unit(u, lc):
                ps = psum.tile([128, 512], F32, tag="mm", bufs=2, name="psqk")
                for kc in range(KC):
                    nc.tensor.matmul(
                        ps,
                        lhsT=w_qk_sb[u][:, kc, :],
                        rhs=xT_sb[kc][:, lc * 512 : (lc + 1) * 512],
                        start=(kc == 0),
                        stop=(kc == KC - 1),
                    )
                for half in range(2):
                    tiles, h, rb = UNIT_DST[(u, half)]
                    dst = tiles[h][:, lc * 512 : (lc + 1) * 512]
                    bias = qkb_sb[:, 2 * u + half : 2 * u + half + 1]
                    mask = qkb_sb[:, 8 + half : 9 + half]
                    # full-width copy: ps*mask + bias zeroes the other
                    # head's 64 rows while writing this head's (the bias
                    # column is zero there), keeping the tile K=128-clean.
                    # Both halves stay on DVE: routing one through ACT
                    # queues it ahead of upcoming exps and stalls the OTs.
                    nc.vector.tensor_scalar(
                        dst,
                        ps,
                        mask,
                        bias,
                        mybir.AluOpType.mult,
                        mybir.AluOpType.add,
                    )

            def emit_v_unit(lt):
                ps = psum.tile([128, HPC, DK], F32, tag="mm", bufs=2, name="psv")
                for kc in range(KC):
                    nc.tensor.matmul(
                        ps,
                        lhsT=xT_sb[kc][:, lt * 128 : (lt + 1) * 128],
                        rhs=w_v_sb[kc],
                        start=(kc == 0),
                        stop=(kc == KC - 1),
                    )
                nc.vector.tensor_add(v_sb[lt][:, :, 0:DK], ps, vb_sb[:, :, 0:DK])

            def emit_outproj_unit(lt):
                yp = psum.tile([128, C], F32, tag="mm", bufs=2, name="psy")
                for pr in range(2):
                    nc.tensor.matmul(
                        yp,
                        lhsT=otp_sb[pr][:, lt * 128 : (lt + 1) * 128],
                        rhs=w_out_sb[:, pr, :],
                        start=(pr == 0),
                        stop=(pr == 1),
                    )
                ysb = work.tile([128, C], F16, tag="ysb", bufs=3, name="ysb")
                nc.vector.tensor_copy(ysb, yp)
                eng = nc.sync if lt % 2 == 0 else nc.scalar
                eng.dma_start(out=y[lt * 128 : (lt + 1) * 128, :], in_=ysb)

            def proj_units(lc, with_v=True):
                u = [(emit_qk_unit, (uu, lc)) for uu in range(HPC)]
                if with_v:
                    u += [(emit_v_unit, (lt,)) for lt in range(4 * lc, 4 * lc + 4)]
                return u

            def emit_attention(qb, feed_early, feed_late):
                nkj = 4 * qb + 4
                npairs = nkj // 2

                def st_exp(h, p):
                    st2 = psum.tile(
                        [128, 1024], F32, tag="st2", bufs=2, name="psst"
                    )
                    for half in range(2):
                        kj = 2 * p + half
                        r = kj - 4 * qb
                        # straddle tiles only feed queries at or beyond
                        # the diagonal: columns [128*r, 512)
                        ws = 128 * r if r > 0 else 0
                        nc.tensor.matmul(
                            st2[:, 512 * half + ws : 512 * (half + 1)],
                            lhsT=kT_sb[h][:, kj * 128 : (kj + 1) * 128],
                            rhs=qT_sb[h][:, qb * 512 + ws : (qb + 1) * 512],
                            start=True,
                            stop=True,
                        )
                    se = work.tile([128, 1024], mm, tag="se", bufs=4, name="se")
                    r0 = 2 * p - 4 * qb
                    ws0 = 128 * r0 if r0 > 0 else 0
                    # one exp per pair, spanning both halves. On straddle
                    # pairs this reads the unwritten PSUM strip between the
                    # two right-aligned halves; the resulting garbage lands
                    # in an se region no OT consumes (HW-harmless; CoreSim's
                    # uninit checker rejects it -- the per-instruction ACT
                    # overhead of splitting costs more than the dead cols).
                    nc.scalar.activation(
                        out=se[:, ws0:1024],
                        in_=st2[:, ws0:1024],
                        func=mybir.ActivationFunctionType.Exp,
                        scale=float(SCALE),
                    )
                    return se

                def mask_ot(h, p, se, ot):
                    r0 = 2 * p - 4 * qb
                    if r0 >= 0:
                        # straddle pair: both halves carry a diagonal
                        # 128x128 block needing the triangle mask. The two
                        # blocks sit 640 columns apart in se -- one 3-dim
                        # DVE multiply covers both.
                        ws = 128 * r0 if r0 > 0 else 0
                        base = se[:, ws : ws + 128]
                        se2 = bass.AP(
                            tensor=base.tensor,
                            offset=base.offset,
                            ap=[base.ap[0], [640, 2], [1, 128]],
                        )
                        tri_ap = tri_sb[:, :]
                        tri2 = bass.AP(
                            tensor=tri_ap.tensor,
                            offset=tri_ap.offset,
                            ap=[tri_ap.ap[0], [0, 2], [1, 128]],
                        )
                        nc.vector.tensor_mul(se2, se2, tri2)
                    for half in range(2):
                        kj = 2 * p + half
                        r = kj - 4 * qb
                        ws = 128 * r if r > 0 else 0
                        o = 512 * half
                        nc.tensor.matmul(
                            ot[:, ws:512],
                            lhsT=v_sb[kj][:, h, :],
                            rhs=se[:, o + ws : o + 512],
                            start=(kj == 0),
                            stop=(kj == nkj - 1),
                        )

                def epilogue_a(h, ot):
                    # normalize: ot[:DK] /= ot[DK] -- all off the PE queue:
                    # DVE copy + fast reciprocal, GpSimd partition
                    # broadcast. (The copy must NOT go to ACT: it would
                    # queue ahead of upcoming exps in the ACT FIFO and
                    # stall the OTs.)
                    dnm = work.tile([1, 512], F32, tag="dnm", bufs=2, name="dnm")
                    nc.vector.tensor_copy(dnm, ot[DK : DK + 1, :])
                    rcp = work.tile([1, 512], F32, tag="rcp", bufs=2, name="rcp")
                    nc.vector.reciprocal_approx_fast(out=rcp, in_=dnm)
                    rbs = work.tile([DK, 512], F32, tag="rbs", bufs=2, name="rbs")
                    nc.gpsimd.partition_broadcast(rbs, rcp)
                    return (h, ot, rbs)

                def epilogue_b(h, ot, rbs):
                    # the final multiply is emitted ~2 slots after the
                    # broadcast was issued: if emitted immediately it
                    # head-of-line-blocks the in-order DVE queue waiting on
                    # GpSimd, delaying the projection-feed copies behind it
                    # (whose PSUM bufs the PE then stalls on).
                    rb = 64 * (h % 2)
                    nc.vector.tensor_mul(
                        otp_sb[h // 2][rb : rb + 64, qb * 512 : (qb + 1) * 512],
                        ot[0:DK, :],
                        rbs,
                    )

                # Per-head software pipeline, depth 2: ST/exp leads OT by
                # two pairs within each head. feed_early: one unit per slot
                # until exhausted (data needed soon). feed_late: head-start
                # slots first (the first OT of a head is most exposed to
                # exp latency), remainder spread evenly -- a pair is
                # ACT-heavier (~1.1us exp) than PE-heavy (~0.9us), so
                # clustering feed early would starve the PE at block end.
                nslots = HPC * npairs
                ne = (len(feed_early) + 1) // 2  # early units go 2 per slot
                nl = len(feed_late)
                head_starts = [h * npairs for h in range(HPC) if h * npairs >= ne]
                assigned = set(head_starts[:nl])
                rest = [s for s in range(ne, nslots) if s not in assigned]
                nrem = nl - len(assigned)
                if nrem > 0 and rest:
                    step = len(rest) / nrem
                    for i in range(nrem):
                        assigned.add(rest[min(int(i * step), len(rest) - 1)])
                for h in range(HPC):
                    ot = psum.tile([128, 512], F32, tag="ot", bufs=2, name="psot")
                    se_buf = {}
                    for p in range(min(2, npairs)):
                        se_buf[p] = st_exp(h, p)
                    for p in range(npairs):
                        s = h * npairs + p
                        if feed_early:
                            # up to two per slot: an OT pair consumes two v
                            # tiles, so the early v units must stay ahead
                            for _ in range(2):
                                if feed_early:
                                    fn, args = feed_early.pop(0)
                                    fn(*args)
                        elif s in assigned and feed_late:
                            fn, args = feed_late.pop(0)
                            fn(*args)
                        mask_ot(h, p, se_buf.pop(p), ot)
                        if p + 2 < npairs:
                            se_buf[p + 2] = st_exp(h, p + 2)
                    epilogue_b(*epilogue_a(h, ot))

            # prelude: slice-0 projections, then attention blocks. Feed
            # distribution tracks the PE-vs-ACT balance: early blocks have
            # plenty of projection feed; the final block (32 pair slots, no
            # projections left) gets slice-3's v units early (needed by its
            # own pair 6) plus ALL out-projections for rows 0..11 --
            # without them the PE would run ACT-bound there. Rows 12..15
            # drain at the tail.
            for fn, args in proj_units(0, with_v=False):
                fn(*args)
            for qb in range(QB):
                early, late = [], []
                if qb == 0:
                    # slice-0 v units ride qb0's early feed (2 per slot,
                    # staying ahead of the OT pairs) instead of extending
                    # the serial prelude behind the input-DMA wait
                    early += [(emit_v_unit, (lt,)) for lt in range(0, 4)]
                if qb + 1 < QB:
                    late += proj_units(qb + 1, with_v=(qb + 1 < 3))
                if qb == QB - 1:
                    early += [(emit_v_unit, (lt,)) for lt in range(12, 16)]
                    late += [(emit_outproj_unit, (lt,)) for lt in range(0, 12)]
                emit_attention(qb, early, late)
                for fn, args in early + late:
                    fn(*args)
                early, late = [], []
            # hold the clock-gate warm while the last head's epilogue
            # drains, so the final out-projection runs at 2.4 GHz
            for _ in range(5):
                nc.tensor.matmul(
                    wps, lhsT=warm[:, 0:128], rhs=warm, start=True, stop=True
                )
            for lt in range(12, LT):
                emit_outproj_unit(lt)

    nc.finalize()
    return nc


def _get_nc():
    if MM_MODE not in _CACHE:
        _CACHE[MM_MODE] = _build(MM_MODE)
    return _CACHE[MM_MODE]


def _make_tri():
    # [j, i] = 1 iff i >= j (key j attends-allowed for query i)
    return np.triu(np.ones((128, 128), np.float32))


def kernel(x, W_in, b_in, W_out, b_out):
    x = np.asarray(x, np.float32)
    W_in = np.asarray(W_in, np.float32)
    b_in = np.asarray(b_in, np.float32)
    W_out = np.asarray(W_out, np.float32)
    b_out = np.asarray(b_out, np.float32)

    mmd = _np_mm_dtype()
    tri = _make_tri().astype(mmd)

    in_maps = []
    for c in range(N_CORES):
        b, j = divmod(c, 2)
        w_in_loc = W_in[:, j * 768 : (j + 1) * 768]  # [C, 768]
        b_in_loc = b_in[j * 768 : (j + 1) * 768]  # [768]
        xT = np.ascontiguousarray(x[b].T).astype(mmd)  # [C, L]
        # pack qk columns into M=128 two-head units (see UNIT_DST in _build):
        # unit u halves: (low head = u%2==..) -> [role_h+2 | role_h] with
        # role q for even u, k for odd u, h = u//2
        wq = lambda h: w_in_loc[:, 192 * h : 192 * h + 64]
        wk = lambda h: w_in_loc[:, 192 * h + 64 : 192 * h + 128]
        bq = lambda h: b_in_loc[192 * h : 192 * h + 64]
        bk = lambda h: b_in_loc[192 * h + 64 : 192 * h + 128]
        units = [
            (wq(2), wq(0), bq(2), bq(0)),
            (wk(2), wk(0), bk(2), bk(0)),
            (wq(3), wq(1), bq(3), bq(1)),
            (wk(3), wk(1), bk(3), bk(1)),
        ]
        # w_in_qk: unit-major, partition-major rows ([u, p, kc, d]) so one
        # contiguous DMA delivers a whole unit's weights
        w_in_qk = np.zeros((HPC, 128, KC, 128), np.float32)
        w_in_v = np.zeros((C, HPC, DK), np.float32)
        qkb = np.zeros((128, 10), np.float32)
        for u, (wlo, whi, blo, bhi) in enumerate(units):
            wu = np.concatenate([wlo, whi], axis=1)  # [C, 128]
            w_in_qk[u] = wu.reshape(KC, 128, 128).transpose(1, 0, 2)
            qkb[0:64, 2 * u] = blo
            qkb[64:128, 2 * u + 1] = bhi
        qkb[0:64, 8] = 1.0  # low-half row mask
        qkb[64:128, 9] = 1.0  # high-half row mask
        for h in range(HPC):
            w_in_v[:, h, :] = w_in_loc[:, 192 * h + 128 : 192 * h + 192]
        w_in_qk = np.ascontiguousarray(w_in_qk).astype(mmd)
        w_in_v = np.ascontiguousarray(w_in_v).astype(mmd)
        vb = np.zeros((HPC, DK + 1), np.float32)
        for h in range(HPC):
            vb[h, :DK] = b_in_loc[192 * h + 128 : 192 * h + 192]
        # out-projection weights, head-PAIR packed to match otp_sb: row p of
        # pair pr is W_out row (head 2pr + p//64, dk p%64) of this core's
        # head group.
        w_out_loc = np.empty((128, 2, C), np.float32)
        for pr in range(2):
            for p in range(128):
                hh = j * HPC + 2 * pr + p // 64
                w_out_loc[p, pr, :] = W_out[hh * DK + p % 64, :]
        in_maps.append(
            dict(
                xT=xT,
                w_in_qk=w_in_qk,
                w_in_v=w_in_v,
                qkb=qkb,
                vb=vb,
                w_out=w_out_loc.astype(mmd),
                tri=tri,
            )
        )

    nc = _get_nc()
    res = run_bass_kernel_spmd(
        nc, in_maps, core_ids=list(range(N_CORES)), trace=TRACE
    )
    global LAST_RESULT
    LAST_RESULT = res

    out = np.empty((B, L, C), np.float32)
    for b in range(B):
        out[b] = (
            res.results[2 * b]["y"]
            + res.results[2 * b + 1]["y"]
            + b_out[None, :]
            + x[b]
        )
    return out


# revision 64
# speedup vs baseline: 1.0016x; 1.0016x over previous
"""Trainium2 Bass kernel for nn_AttentionBlock (B=4, L=2048, C=512, H=8, Dk=64).

Sharding (8 cores): data-parallel over B (4) x tensor-parallel over heads (2
groups of 4). Core c handles batch c//2, head group c%2. Each core computes
  y_c = attention(x_b)[:, local_heads] @ W_out[local_rows]        [2048, 512]
and the host combines: out[b] = y[2b] + y[2b+1] + b_out + x[b].

Device kernel (per core); matmul operands fp16, fp32 PSUM accumulation:
  - qT/kT per head in [Dk, L] layout straight out of the projection
    (lhsT=W_in chunk, rhs=xT chunk) -- no transposes anywhere. Each head
    owns a full [128, L] tile whose complementary 64 rows are kept zero
    (mask fused into the projection copy), so every ST matmul is a
    uniform K=128 / (128,128)-tile op: mixing 64- and 128-row weight
    tiles costs a ~90ns PE array-reconfig stall per switch.
  - v in natural [L, Dk] layout, augmented with a ones column (plus zero
    padding to 128, written once at startup) so the O^T = V^T P^T matmul
    also produces the softmax denominators for free.
  - scores S^T [keys, queries]; causal structure skips upper-triangle
    tiles and narrows diagonal-straddling tiles; both diagonal 128x128
    blocks of a straddle pair get one batched 0/1 triangle multiply
    (3-dim AP, 640-col stride) after exp.
  - exp batched over key-tile pairs (one 2-bank PSUM tile), split per
    half on straddle pairs so only written PSUM is read; the
    ST->exp->OT chain is software-pipelined 2 pairs deep so the OT
    matmul never waits on ACT latency. ACT runs exp ONLY -- any copy
    routed through it queues ahead of exps and stalls the PE.
  - softmax normalization off the PE queue: DVE copy (custom-DVE ops
    must not read PSUM directly on HW) + fast reciprocal, GpSimd
    partition-broadcast, DVE multiply writing into a head-PAIR packed
    layout (head 2p in partitions 0:64, 2p+1 in 64:128) so the
    out-projection runs K=128 matmuls (2 per row tile, not 4).
  - emission order keeps the (strictly in-order) PE queue dense: a
    warmup burst covers the input-DMA wait + HAM clock ramp;
    projection/out-projection units are interleaved between attention
    pairs with a block-level budget matched to the ACT-vs-PE balance
    (the final block gets all deferred out-projections); input DMA is
    spread across three engine queues with the startup-critical slices
    (xT cols 0:512, unit-major q/k weights) first.
fp16 operands keep absmax-relative error vs the fp32 reference at ~4e-4
(8x tighter than bf16) at identical PE throughput; y returned as fp16.
"""

import sys

sys.path.insert(0, "/opt/trn_rl_repo")

import numpy as np

import concourse.bacc as bacc
import concourse.bass as bass
import concourse.mybir as mybir
import concourse.tile as tile
from concourse.bass_utils import run_bass_kernel_spmd

# ---------------------------------------------------------------- constants
B, L, C = 4, 2048, 512
H, DK = 8, 64
HPC = 4  # heads per core
SCALE = DK**-0.5
N_CORES = 8
KC = C // 128  # 4 contraction chunks
LT = L // 128  # 16 row tiles
QB = L // 512  # 4 query blocks of 512

F32 = mybir.dt.float32
BF16 = mybir.dt.bfloat16
F16 = mybir.dt.float16

# matmul operand dtype: "fp16" (fast, accurate) / "bf16" / "fp32" (exact)
MM_MODE = "fp16"

# test hooks (grading path leaves these alone)
TRACE = False
LAST_RESULT = None

_CACHE = {}


def _np_mm_dtype():
    if MM_MODE == "bf16":
        import ml_dtypes

        return ml_dtypes.bfloat16
    if MM_MODE == "fp16":
        return np.float16
    return np.float32


def _mm_dt():
    return {"bf16": BF16, "fp16": F16, "fp32": F32}[MM_MODE]


def _build(mm_mode):
    mm = {"bf16": BF16, "fp16": F16, "fp32": F32}[mm_mode]
    nc = bacc.Bacc(None)

    xT = nc.declare_dram_parameter("xT", [C, L], mm, isOutput=False)
    # unit-major (and pre-transposed to partition-major rows) so each
    # unit's weights arrive in one contiguous DMA, letting the prelude
    # start as soon as the first unit's slice lands
    w_in_qk = nc.declare_dram_parameter("w_in_qk", [HPC, 128, KC, 128], mm, isOutput=False)
    w_in_v = nc.declare_dram_parameter("w_in_v", [C, HPC, DK], mm, isOutput=False)
    # cols 0..7: q/k biases per (unit, half); col 8: low-half row mask
    # (partitions 0:64), col 9: high-half row mask (64:128)
    qkb = nc.declare_dram_parameter("qkb", [128, 10], F32, isOutput=False)
    vb = nc.declare_dram_parameter("vb", [HPC, DK + 1], F32, isOutput=False)
    w_out = nc.declare_dram_parameter("w_out", [128, 2, C], mm, isOutput=False)
    tri = nc.declare_dram_parameter("tri", [128, 128], mm, isOutput=False)
    y = nc.declare_dram_parameter("y", [L, C], F16, isOutput=True)

    with tile.TileContext(nc) as tc:
        with (
            tc.tile_pool(name="persist", bufs=1) as per,
            tc.tile_pool(name="work", bufs=2) as work,
            tc.tile_pool(name="psum", bufs=1, space="PSUM") as psum,
        ):
            # ---------------- loads
            xT_sb = [per.tile([128, L], mm, tag=f"xT{i}", name=f"xT{i}") for i in range(KC)]
            w_qk_sb = [per.tile([128, KC, 128], mm, tag=f"wq{u}", name=f"wq{u}") for u in range(HPC)]
            w_v_sb = [per.tile([128, HPC, DK], mm, tag=f"wv{i}", name=f"wv{i}") for i in range(KC)]
            w_out_sb = per.tile([128, 2, C], mm, tag="wo")
            tri_sb = per.tile([128, 128], mm, tag="tri")
            qkb_sb = per.tile([128, 10], F32, tag="qkb")
            vb_sb = per.tile([128, HPC, DK + 1], F32, tag="vb")

            # PE warmup: dependency-free dummy matmuls fill the input-DMA
            # wait and hold the HAM clock-gate warm before real work starts
            # (otherwise warm/cold entry is start-phase luck, ~+30us).
            warm = per.tile([128, 512], mm, tag="warm")
            nc.vector.memset(warm, 0.0)
            wps = psum.tile([128, 512], F32, tag="ot", bufs=2, name="warmps")
            for _ in range(10):
                nc.tensor.matmul(
                    wps, lhsT=warm[:, 0:128], rhs=warm, start=True, stop=True
                )

            # Input loads: DMA issue is ~0.6us per dma_start per engine
            # queue and each queue sustains ~100 GB/s, so spread the load
            # across four engine queues. The first projection unit needs
            # cols 0:512 of ALL four xT chunks plus the q/k weights (the
            # startup critical path), so those go first on their queues;
            # v weights ride the otherwise-idle vector queue.
            xT_t = xT.rearrange("(c p) l -> c p l", p=128)
            w_v_t = w_in_v.rearrange("(c p) h d -> c p h d", p=128)
            for i in range(KC):
                eng = nc.sync if i < 2 else nc.scalar
                eng.dma_start(out=xT_sb[i][:, 0:512], in_=xT_t[i][:, 0:512])
                nc.gpsimd.dma_start(out=w_qk_sb[i], in_=w_in_qk[i])
            for i in range(KC):
                nc.scalar.dma_start(out=w_v_sb[i], in_=w_v_t[i])
            nc.sync.dma_start(out=qkb_sb, in_=qkb[:, :])
            vb_ap = vb[:, :]
            vb_bcast = bass.AP(
                tensor=vb_ap.tensor, offset=vb_ap.offset, ap=[[0, 128], *vb_ap.ap]
            )
            nc.sync.dma_start(out=vb_sb, in_=vb_bcast)
            nc.sync.dma_start(out=tri_sb, in_=tri[:, :])
            for i in range(KC):
                eng = nc.sync if i < 2 else nc.scalar
                eng.dma_start(out=xT_sb[i][:, 512:L], in_=xT_t[i][:, 512:L])
            nc.scalar.dma_start(out=w_out_sb, in_=w_out[:, :, :])

            # ---------------- fused pipeline ----------------
            # Attention per key-tile pair: ST matmuls -> ACT exp -> OT
            # matmuls, software-pipelined two pairs deep; projection and
            # out-projection matmuls are fed into the PE queue one unit at
            # a time between pairs so the PE stays dense while ACT works.
            # heads 0,1 keep q/k in partitions 64:128; heads 2,3 in 0:64 --
            # one M=128 projection matmul serves two heads (host packs W_in
            # columns accordingly). Each head owns a full [128, L] tile whose
            # complementary 64 rows are KEPT ZERO (the projection copy
            # multiplies by a per-partition row mask), so every ST matmul is
            # a full K=128 / (128,128)-tile op -- mixing 64-row and 128-row
            # weight tiles on the PE costs an array-reconfig stall per
            # switch.
            qT_sb = [per.tile([128, L], mm, tag=f"qT{h}", name=f"qT{h}") for h in range(HPC)]
            kT_sb = [per.tile([128, L], mm, tag=f"kT{h}", name=f"kT{h}") for h in range(HPC)]

            # (unit, psum-half) -> (role tiles, head, tile row base)
            UNIT_DST = {
                (0, 0): (qT_sb, 2, 0), (0, 1): (qT_sb, 0, 64),
                (1, 0): (kT_sb, 2, 0), (1, 1): (kT_sb, 0, 64),
                (2, 0): (qT_sb, 3, 0), (2, 1): (qT_sb, 1, 64),
                (3, 0): (kT_sb, 3, 0), (3, 1): (kT_sb, 1, 64),
            }
            v_sb = [per.tile([128, HPC, 128], mm, tag=f"v{lt}", name=f"v{lt}") for lt in range(LT)]
            # constant regions of the v tiles (zero pad for fast weight
            # load + the ones column that produces softmax denominators):
            # written once here, in the input-DMA wait window, instead of
            # per v-unit in steady state
            for lt in range(LT):
                nc.vector.memset(v_sb[lt][:, :, DK:128], 0.0)
                nc.vector.memset(v_sb[lt][:, :, DK : DK + 1], 1.0)
            # attention output, head-PAIR packed: pair p holds head 2p in
            # partitions 0:64 and head 2p+1 in 64:128 -> out-projection
            # contracts K=128 (two heads per matmul).
            otp_sb = [per.tile([128, L], mm, tag=f"otp{p}", name=f"otp{p}") for p in range(2)]

            def emit_qk_unit(u, lc):
                ps = psum.tile([128, 512], F32, tag="mm", bufs=2, name="psqk")
                for kc in range(KC):
                    nc.tensor.matmul(
                        ps,
                        lhsT=w_qk_sb[u][:, kc, :],
                        rhs=xT_sb[kc][:, lc * 512 : (lc + 1) * 512],
                        start=(kc == 0),
                        stop=(kc == KC - 1),
                    )
                for half in range(2):
                    tiles, h, rb = UNIT_DST[(u, half)]
                    dst = tiles[h][:, lc * 512 : (lc + 1) * 512]
                    bias = qkb_sb[:, 2 * u + half : 2 * u + half + 1]
                    mask = qkb_sb[:, 8 + half : 9 + half]
                    # full-width copy: ps*mask + bias zeroes the other
                    # head's 64 rows while writing this head's (the bias
                    # column is zero there), keeping the tile K=128-clean.
                    # Both halves stay on DVE: routing one through ACT
                    # queues it ahead of upcoming exps and stalls the OTs.
                    nc.vector.tensor_scalar(
                        dst,
                        ps,
                        mask,
                        bias,
                        mybir.AluOpType.mult,
                        mybir.AluOpType.add,
                    )

            def emit_v_unit(lt):
                ps = psum.tile([128, HPC, DK], F32, tag="mm", bufs=2, name="psv")
                for kc in range(KC):
                    nc.tensor.matmul(
                        ps,
                        lhsT=xT_sb[kc][:, lt * 128 : (lt + 1) * 128],
                        rhs=w_v_sb[kc],
                        start=(kc == 0),
                        stop=(kc == KC - 1),
                    )
                nc.vector.tensor_add(v_sb[lt][:, :, 0:DK], ps, vb_sb[:, :, 0:DK])

            def emit_outproj_unit(lt):
                yp = psum.tile([128, C], F32, tag="mm", bufs=2, name="psy")
                for pr in range(2):
                    nc.tensor.matmul(
                        yp,
                        lhsT=otp_sb[pr][:, lt * 128 : (lt + 1) * 128],
                        rhs=w_out_sb[:, pr, :],
                        start=(pr == 0),
                        stop=(pr == 1),
                    )
                ysb = work.tile([128, C], F16, tag="ysb", bufs=3, name="ysb")
                nc.vector.tensor_copy(ysb, yp)
                eng = nc.sync if lt % 2 == 0 else nc.scalar
                eng.dma_start(out=y[lt * 128 : (lt + 1) * 128, :], in_=ysb)

            def proj_units(lc, with_v=True):
                u = [(emit_qk_unit, (uu, lc)) for uu in range(HPC)]
                if with_v:
                    u += [(emit_v_unit, (lt,)) for lt in range(4 * lc, 4 * lc + 4)]
                return u

            def emit_attention(qb, feed_early, feed_late):
                nkj = 4 * qb + 4
                npairs = nkj // 2

                def st_exp(h, p):
                    st2 = psum.tile(
                        [128, 1024], F32, tag="st2", bufs=2, name="psst"
                    )
                    for half in range(2):
                        kj = 2 * p + half
                        r = kj - 4 * qb
                        # straddle tiles only feed queries at or beyond
                        # the diagonal: columns [128*r, 512)
                        ws = 128 * r if r > 0 else 0
                        nc.tensor.matmul(
                            st2[:, 512 * half + ws : 512 * (half + 1)],
                            lhsT=kT_sb[h][:, kj * 128 : (kj + 1) * 128],
                            rhs=qT_sb[h][:, qb * 512 + ws : (qb + 1) * 512],
                            start=True,
                            stop=True,
                        )
                    se = work.tile([128, 1024], mm, tag="se", bufs=4, name="se")
                    r0 = 2 * p - 4 * qb
                    ws0 = 128 * r0 if r0 > 0 else 0
                    # one exp per pair, spanning both halves. On straddle
                    # pairs this reads the unwritten PSUM strip between the
                    # two right-aligned halves; the garbage lands in an se
                    # region no OT consumes (HW-harmless; CoreSim's uninit
                    # checker rejects it -- split per half there instead).
                    nc.scalar.activation(
                        out=se[:, ws0:1024],
                        in_=st2[:, ws0:1024],
                        func=mybir.ActivationFunctionType.Exp,
                        scale=float(SCALE),
                    )
                    return se

                def mask_ot(h, p, se, ot):
                    r0 = 2 * p - 4 * qb
                    if r0 >= 0:
                        # straddle pair: both halves carry a diagonal
                        # 128x128 block needing the triangle mask. The two
                        # blocks sit 640 columns apart in se -- one 3-dim
                        # DVE multiply covers both.
                        ws = 128 * r0 if r0 > 0 else 0
                        base = se[:, ws : ws + 128]
                        se2 = bass.AP(
                            tensor=base.tensor,
                            offset=base.offset,
                            ap=[base.ap[0], [640, 2], [1, 128]],
                        )
                        tri_ap = tri_sb[:, :]
                        tri2 = bass.AP(
                            tensor=tri_ap.tensor,
                            offset=tri_ap.offset,
                            ap=[tri_ap.ap[0], [0, 2], [1, 128]],
                        )
                        nc.vector.tensor_mul(se2, se2, tri2)
                    for half in range(2):
                        kj = 2 * p + half
                        r = kj - 4 * qb
                        ws = 128 * r if r > 0 else 0
                        o = 512 * half
                        nc.tensor.matmul(
                            ot[:, ws:512],
                            lhsT=v_sb[kj][:, h, :],
                            rhs=se[:, o + ws : o + 512],
                            start=(kj == 0),
                            stop=(kj == nkj - 1),
                        )

                def epilogue_a(h, ot):
                    # normalize: ot[:DK] /= ot[DK] -- all off the PE queue:
                    # DVE copy + fast reciprocal, GpSimd partition
                    # broadcast. (The copy must NOT go to ACT: it would
                    # queue ahead of upcoming exps in the ACT FIFO and
                    # stall the OTs.)
                    dnm = work.tile([1, 512], F32, tag="dnm", bufs=2, name="dnm")
                    nc.vector.tensor_copy(dnm, ot[DK : DK + 1, :])
                    rcp = work.tile([1, 512], F32, tag="rcp", bufs=2, name="rcp")
                    nc.vector.reciprocal_approx_fast(out=rcp, in_=dnm)
                    rbs = work.tile([DK, 512], F32, tag="rbs", bufs=2, name="rbs")
                    nc.gpsimd.partition_broadcast(rbs, rcp)
                    return (h, ot, rbs)

                def epilogue_b(h, ot, rbs):
                    # the final multiply is emitted ~2 slots after the
                    # broadcast was issued: if emitted immediately it
                    # head-of-line-blocks the in-order DVE queue waiting on
                    # GpSimd, delaying the projection-feed copies behind it
                    # (whose PSUM bufs the PE then stalls on).
                    rb = 64 * (h % 2)
                    nc.vector.tensor_mul(
                        otp_sb[h // 2][rb : rb + 64, qb * 512 : (qb + 1) * 512],
                        ot[0:DK, :],
                        rbs,
                    )

                # Per-head software pipeline, depth 2: ST/exp leads OT by
                # two pairs within each head. feed_early: one unit per slot
                # until exhausted (data needed soon). feed_late: head-start
                # slots first (the first OT of a head is most exposed to
                # exp latency), remainder spread evenly -- a pair is
                # ACT-heavier (~1.1us exp) than PE-heavy (~0.9us), so
                # clustering feed early would starve the PE at block end.
                nslots = HPC * npairs
                ne = (len(feed_early) + 1) // 2  # early units go 2 per slot
                nl = len(feed_late)
                head_starts = [h * npairs for h in range(HPC) if h * npairs >= ne]
                assigned = set(head_starts[:nl])
                rest = [s for s in range(ne, nslots) if s not in assigned]
                nrem = nl - len(assigned)
                if nrem > 0 and rest:
                    step = len(rest) / nrem
                    for i in range(nrem):
                        assigned.add(rest[min(int(i * step), len(rest) - 1)])
                for h in range(HPC):
                    ot = psum.tile([128, 512], F32, tag="ot", bufs=2, name="psot")
                    se_buf = {}
                    for p in range(min(2, npairs)):
                        se_buf[p] = st_exp(h, p)
                    for p in range(npairs):
                        s = h * npairs + p
                        if feed_early:
                            # up to two per slot: an OT pair consumes two v
                            # tiles, so the early v units must stay ahead
                            for _ in range(2):
                                if feed_early:
                                    fn, args = feed_early.pop(0)
                                    fn(*args)
                        elif s in assigned and feed_late:
                            fn, args = feed_late.pop(0)
                            fn(*args)
                        mask_ot(h, p, se_buf.pop(p), ot)
                        if p + 2 < npairs:
                            se_buf[p + 2] = st_exp(h, p + 2)
                    epilogue_b(*epilogue_a(h, ot))

            # prelude: slice-0 projections, then attention blocks. Feed
            # distribution tracks the PE-vs-ACT balance: early blocks have
            # plenty of projection feed; the final block (32 pair slots, no
            # projections left) gets slice-3's v units early (needed by its
            # own pair 6) plus ALL out-projections for rows 0..11 --
            # without them the PE would run ACT-bound there. Rows 12..15
            # drain at the tail.
            for fn, args in proj_units(0, with_v=False):
                fn(*args)
            for qb in range(QB):
                early, late = [], []
                if qb == 0:
                    # slice-0 v units ride qb0's early feed (2 per slot,
                    # staying ahead of the OT pairs) instead of extending
                    # the serial prelude behind the input-DMA wait
                    early += [(emit_v_unit, (lt,)) for lt in range(0, 4)]
                if qb + 1 < QB:
                    late += proj_units(qb + 1, with_v=(qb + 1 < 3))
                if qb == QB - 1:
                    early += [(emit_v_unit, (lt,)) for lt in range(12, 16)]
                    late += [(emit_outproj_unit, (lt,)) for lt in range(0, 12)]
                emit_attention(qb, early, late)
                for fn, args in early + late:
                    fn(*args)
                early, late = [], []
            # hold the clock-gate warm while the last head's epilogue
            # drains, so the final out-projection runs at 2.4 GHz
            for _ in range(5):
                nc.tensor.matmul(
                    wps, lhsT=warm[:, 0:128], rhs=warm, start=True, stop=True
                )
            for lt in range(12, LT):
                emit_outproj_unit(lt)

    nc.finalize()
    return nc


def _get_nc():
    if MM_MODE not in _CACHE:
        _CACHE[MM_MODE] = _build(MM_MODE)
    return _CACHE[MM_MODE]


def _make_tri():
    # [j, i] = 1 iff i >= j (key j attends-allowed for query i)
    return np.triu(np.ones((128, 128), np.float32))


def kernel(x, W_in, b_in, W_out, b_out):
    x = np.asarray(x, np.float32)
    W_in = np.asarray(W_in, np.float32)
    b_in = np.asarray(b_in, np.float32)
    W_out = np.asarray(W_out, np.float32)
    b_out = np.asarray(b_out, np.float32)

    mmd = _np_mm_dtype()
    tri = _make_tri().astype(mmd)

    in_maps = []
    for c in range(N_CORES):
        b, j = divmod(c, 2)
        w_in_loc = W_in[:, j * 768 : (j + 1) * 768]  # [C, 768]
        b_in_loc = b_in[j * 768 : (j + 1) * 768]  # [768]
        xT = np.ascontiguousarray(x[b].T).astype(mmd)  # [C, L]
        # pack qk columns into M=128 two-head units (see UNIT_DST in _build):
        # unit u halves: (low head = u%2==..) -> [role_h+2 | role_h] with
        # role q for even u, k for odd u, h = u//2
        wq = lambda h: w_in_loc[:, 192 * h : 192 * h + 64]
        wk = lambda h: w_in_loc[:, 192 * h + 64 : 192 * h + 128]
        bq = lambda h: b_in_loc[192 * h : 192 * h + 64]
        bk = lambda h: b_in_loc[192 * h + 64 : 192 * h + 128]
        units = [
            (wq(2), wq(0), bq(2), bq(0)),
            (wk(2), wk(0), bk(2), bk(0)),
            (wq(3), wq(1), bq(3), bq(1)),
            (wk(3), wk(1), bk(3), bk(1)),
        ]
        # w_in_qk: unit-major, partition-major rows ([u, p, kc, d]) so one
        # contiguous DMA delivers a whole unit's weights
        w_in_qk = np.zeros((HPC, 128, KC, 128), np.float32)
        w_in_v = np.zeros((C, HPC, DK), np.float32)
        qkb = np.zeros((128, 10), np.float32)
        for u, (wlo, whi, blo, bhi) in enumerate(units):
            wu = np.concatenate([wlo, whi], axis=1)  # [C, 128]
            w_in_qk[u] = wu.reshape(KC, 128, 128).transpose(1, 0, 2)
            qkb[0:64, 2 * u] = blo
            qkb[64:128, 2 * u + 1] = bhi
        qkb[0:64, 8] = 1.0  # low-half row mask
        qkb[64:128, 9] = 1.0  # high-half row mask
        for h in range(HPC):
            w_in_v[:, h, :] = w_in_loc[:, 192 * h + 128 : 192 * h + 192]
        w_in_qk = np.ascontiguousarray(w_in_qk).astype(mmd)
        w_in_v = np.ascontiguousarray(w_in_v).astype(mmd)
        vb = np.zeros((HPC, DK + 1), np.float32)
        for h in range(HPC):
            vb[h, :DK] = b_in_loc[192 * h + 128 : 192 * h + 192]
        # out-projection weights, head-PAIR packed to match otp_sb: row p of
        # pair pr is W_out row (head 2pr + p//64, dk p%64) of this core's
        # head group.
        w_out_loc = np.empty((128, 2, C), np.float32)
        for pr in range(2):
            for p in range(128):
                hh = j * HPC + 2 * pr + p // 64
                w_out_loc[p, pr, :] = W_out[hh * DK + p % 64, :]
        in_maps.append(
            dict(
                xT=xT,
                w_in_qk=w_in_qk,
                w_in_v=w_in_v,
                qkb=qkb,
                vb=vb,
                w_out=w_out_loc.astype(mmd),
                tri=tri,
            )
        )

    nc = _get_nc()
    res = run_bass_kernel_spmd(
        nc, in_maps, core_ids=list(range(N_CORES)), trace=TRACE
    )
    global LAST_RESULT
    LAST_RESULT = res

    out = np.empty((B, L, C), np.float32)
    for b in range(B):
        out[b] = (
            res.results[2 * b]["y"]
            + res.results[2 * b + 1]["y"]
            + b_out[None, :]
            + x[b]
        )
    return out


# revision 65
# speedup vs baseline: 1.1911x; 1.1893x over previous
"""Trainium2 Bass kernel for nn_AttentionBlock (B=4, L=2048, C=512, H=8, Dk=64).

Sharding (8 cores): data-parallel over B (4) x tensor-parallel over heads (2
groups of 4). Core c handles batch c//2, head group c%2. Each core computes
  y_c = attention(x_b)[:, local_heads] @ W_out[local_rows]        [2048, 512]
and the host combines: out[b] = y[2b] + y[2b+1] + b_out + x[b].

Device kernel (per core); matmul operands fp16, fp32 PSUM accumulation:
  - qT/kT per head in [Dk, L] layout straight out of the projection
    (lhsT=W_in chunk, rhs=xT chunk) -- no transposes anywhere. Each head
    owns a full [128, L] tile whose complementary 64 rows are kept zero
    (mask fused into the projection copy), so every ST matmul is a
    uniform K=128 / (128,128)-tile op: mixing 64- and 128-row weight
    tiles costs a ~90ns PE array-reconfig stall per switch.
  - v in natural [L, Dk] layout, augmented with a ones column (plus zero
    padding to 128, written once at startup) so the O^T = V^T P^T matmul
    also produces the softmax denominators for free.
  - scores S^T [keys, queries]; causal structure skips upper-triangle
    tiles and narrows diagonal-straddling tiles; both diagonal 128x128
    blocks of a straddle pair get one batched 0/1 triangle multiply
    (3-dim AP, 640-col stride) after exp.
  - exp batched over key-tile pairs (one 2-bank PSUM tile), split per
    half on straddle pairs so only written PSUM is read; the
    ST->exp->OT chain is software-pipelined 2 pairs deep so the OT
    matmul never waits on ACT latency. ACT runs exp ONLY -- any copy
    routed through it queues ahead of exps and stalls the PE.
  - softmax normalization off the PE queue: DVE copy (custom-DVE ops
    must not read PSUM directly on HW) + fast reciprocal, GpSimd
    partition-broadcast, DVE multiply writing into a head-PAIR packed
    layout (head 2p in partitions 0:64, 2p+1 in 64:128) so the
    out-projection runs K=128 matmuls (2 per row tile, not 4).
  - emission order keeps the (strictly in-order) PE queue dense: a
    warmup burst covers the input-DMA wait + HAM clock ramp;
    projection/out-projection units are interleaved between attention
    pairs with a block-level budget matched to the ACT-vs-PE balance
    (the final block gets all deferred out-projections); input DMA is
    spread across three engine queues with the startup-critical slices
    (xT cols 0:512, unit-major q/k weights) first.
fp16 operands keep absmax-relative error vs the fp32 reference at ~4e-4
(8x tighter than bf16) at identical PE throughput; y returned as fp16.
"""

import sys

sys.path.insert(0, "/opt/trn_rl_repo")

import numpy as np

import concourse.bacc as bacc
import concourse.bass as bass
import concourse.mybir as mybir
import concourse.tile as tile
from concourse.bass_utils import run_bass_kernel_spmd

# ---------------------------------------------------------------- constants
B, L, C = 4, 2048, 512
H, DK = 8, 64
HPC = 4  # heads per core
SCALE = DK**-0.5
N_CORES = 8
KC = C // 128  # 4 contraction chunks
LT = L // 128  # 16 row tiles
QB = L // 512  # 4 query blocks of 512

F32 = mybir.dt.float32
BF16 = mybir.dt.bfloat16
F16 = mybir.dt.float16

# matmul operand dtype: "fp16" (fast, accurate) / "bf16" / "fp32" (exact)
MM_MODE = "fp16"

# test hooks (grading path leaves these alone)
TRACE = False
LAST_RESULT = None

_CACHE = {}


def _np_mm_dtype():
    if MM_MODE == "bf16":
        import ml_dtypes

        return ml_dtypes.bfloat16
    if MM_MODE == "fp16":
        return np.float16
    return np.float32


def _mm_dt():
    return {"bf16": BF16, "fp16": F16, "fp32": F32}[MM_MODE]


def _build(mm_mode):
    mm = {"bf16": BF16, "fp16": F16, "fp32": F32}[mm_mode]
    nc = bacc.Bacc(None)

    xT = nc.declare_dram_parameter("xT", [C, L], mm, isOutput=False)
    # unit-major (and pre-transposed to partition-major rows) so each
    # unit's weights arrive in one contiguous DMA, letting the prelude
    # start as soon as the first unit's slice lands
    w_in_qk = nc.declare_dram_parameter("w_in_qk", [HPC, 128, KC, 128], mm, isOutput=False)
    w_in_v = nc.declare_dram_parameter("w_in_v", [C, HPC, DK], mm, isOutput=False)
    # cols 0..7: q/k biases per (unit, half); col 8: low-half row mask
    # (partitions 0:64), col 9: high-half row mask (64:128)
    qkb = nc.declare_dram_parameter("qkb", [128, 10], F32, isOutput=False)
    vb = nc.declare_dram_parameter("vb", [HPC, DK + 1], F32, isOutput=False)
    w_out = nc.declare_dram_parameter("w_out", [128, 2, C], mm, isOutput=False)
    tri = nc.declare_dram_parameter("tri", [128, 128], mm, isOutput=False)
    y = nc.declare_dram_parameter("y", [L, C], F16, isOutput=True)

    with tile.TileContext(nc) as tc:
        with (
            tc.tile_pool(name="persist", bufs=1) as per,
            tc.tile_pool(name="work", bufs=2) as work,
            tc.tile_pool(name="psum", bufs=1, space="PSUM") as psum,
        ):
            # ---------------- loads
            xT_sb = [per.tile([128, L], mm, tag=f"xT{i}", name=f"xT{i}") for i in range(KC)]
            w_qk_sb = [per.tile([128, KC, 128], mm, tag=f"wq{u}", name=f"wq{u}") for u in range(HPC)]
            w_v_sb = [per.tile([128, HPC, DK], mm, tag=f"wv{i}", name=f"wv{i}") for i in range(KC)]
            w_out_sb = per.tile([128, 2, C], mm, tag="wo")
            tri_sb = per.tile([128, 128], mm, tag="tri")
            qkb_sb = per.tile([128, 10], F32, tag="qkb")
            vb_sb = per.tile([128, HPC, DK + 1], F32, tag="vb")

            # PE warmup: dependency-free dummy matmuls fill the input-DMA
            # wait and hold the HAM clock-gate warm before real work starts
            # (otherwise warm/cold entry is start-phase luck, ~+30us).
            warm = per.tile([128, 512], mm, tag="warm")
            nc.vector.memset(warm, 0.0)
            wps = psum.tile([128, 512], F32, tag="ot", bufs=2, name="warmps")
            for _ in range(10):
                nc.tensor.matmul(
                    wps, lhsT=warm[:, 0:128], rhs=warm, start=True, stop=True
                )

            # Input loads: DMA issue is ~0.6us per dma_start per engine
            # queue and each queue sustains ~100 GB/s, so spread the load
            # across four engine queues. The first projection unit needs
            # cols 0:512 of ALL four xT chunks plus the q/k weights (the
            # startup critical path), so those go first on their queues;
            # v weights ride the otherwise-idle vector queue.
            xT_t = xT.rearrange("(c p) l -> c p l", p=128)
            w_v_t = w_in_v.rearrange("(c p) h d -> c p h d", p=128)
            for i in range(KC):
                eng = nc.sync if i < 2 else nc.scalar
                eng.dma_start(out=xT_sb[i][:, 0:512], in_=xT_t[i][:, 0:512])
                nc.gpsimd.dma_start(out=w_qk_sb[i], in_=w_in_qk[i])
            for i in range(KC):
                nc.scalar.dma_start(out=w_v_sb[i], in_=w_v_t[i])
            nc.sync.dma_start(out=qkb_sb, in_=qkb[:, :])
            vb_ap = vb[:, :]
            vb_bcast = bass.AP(
                tensor=vb_ap.tensor, offset=vb_ap.offset, ap=[[0, 128], *vb_ap.ap]
            )
            nc.sync.dma_start(out=vb_sb, in_=vb_bcast)
            nc.sync.dma_start(out=tri_sb, in_=tri[:, :])
            for i in range(KC):
                eng = nc.sync if i < 2 else nc.scalar
                eng.dma_start(out=xT_sb[i][:, 512:L], in_=xT_t[i][:, 512:L])
            nc.scalar.dma_start(out=w_out_sb, in_=w_out[:, :, :])

            # ---------------- fused pipeline ----------------
            # Attention per key-tile pair: ST matmuls -> ACT exp -> OT
            # matmuls, software-pipelined two pairs deep; projection and
            # out-projection matmuls are fed into the PE queue one unit at
            # a time between pairs so the PE stays dense while ACT works.
            # heads 0,1 keep q/k in partitions 64:128; heads 2,3 in 0:64 --
            # one M=128 projection matmul serves two heads (host packs W_in
            # columns accordingly). Each head owns a full [128, L] tile whose
            # complementary 64 rows are KEPT ZERO (the projection copy
            # multiplies by a per-partition row mask), so every ST matmul is
            # a full K=128 / (128,128)-tile op -- mixing 64-row and 128-row
            # weight tiles on the PE costs an array-reconfig stall per
            # switch.
            qT_sb = [per.tile([128, L], mm, tag=f"qT{h}", name=f"qT{h}") for h in range(HPC)]
            kT_sb = [per.tile([128, L], mm, tag=f"kT{h}", name=f"kT{h}") for h in range(HPC)]

            # (unit, psum-half) -> (role tiles, head, tile row base)
            UNIT_DST = {
                (0, 0): (qT_sb, 2, 0), (0, 1): (qT_sb, 0, 64),
                (1, 0): (kT_sb, 2, 0), (1, 1): (kT_sb, 0, 64),
                (2, 0): (qT_sb, 3, 0), (2, 1): (qT_sb, 1, 64),
                (3, 0): (kT_sb, 3, 0), (3, 1): (kT_sb, 1, 64),
            }
            v_sb = [per.tile([128, HPC, 128], mm, tag=f"v{lt}", name=f"v{lt}") for lt in range(LT)]
            # constant regions of the v tiles (zero pad for fast weight
            # load + the ones column that produces softmax denominators):
            # written once here, in the input-DMA wait window, instead of
            # per v-unit in steady state
            for lt in range(LT):
                nc.vector.memset(v_sb[lt][:, :, DK:128], 0.0)
                nc.vector.memset(v_sb[lt][:, :, DK : DK + 1], 1.0)
            # attention output, head-PAIR packed: pair p holds head 2p in
            # partitions 0:64 and head 2p+1 in 64:128 -> out-projection
            # contracts K=128 (two heads per matmul).
            otp_sb = [per.tile([128, L], mm, tag=f"otp{p}", name=f"otp{p}") for p in range(2)]

            def emit_qk_unit(u, lc):
                ps = psum.tile([128, 512], F32, tag="mm", bufs=2, name="psqk")
                for kc in range(KC):
                    nc.tensor.matmul(
                        ps,
                        lhsT=w_qk_sb[u][:, kc, :],
                        rhs=xT_sb[kc][:, lc * 512 : (lc + 1) * 512],
                        start=(kc == 0),
                        stop=(kc == KC - 1),
                    )
                for half in range(2):
                    tiles, h, rb = UNIT_DST[(u, half)]
                    dst = tiles[h][:, lc * 512 : (lc + 1) * 512]
                    bias = qkb_sb[:, 2 * u + half : 2 * u + half + 1]
                    mask = qkb_sb[:, 8 + half : 9 + half]
                    # full-width copy: ps*mask + bias zeroes the other
                    # head's 64 rows while writing this head's (the bias
                    # column is zero there), keeping the tile K=128-clean.
                    # Both halves stay on DVE: routing one through ACT
                    # queues it ahead of upcoming exps and stalls the OTs.
                    nc.vector.tensor_scalar(
                        dst,
                        ps,
                        mask,
                        bias,
                        mybir.AluOpType.mult,
                        mybir.AluOpType.add,
                    )

            def emit_v_unit(lt):
                ps = psum.tile([128, HPC, DK], F32, tag="mm", bufs=2, name="psv")
                for kc in range(KC):
                    nc.tensor.matmul(
                        ps,
                        lhsT=xT_sb[kc][:, lt * 128 : (lt + 1) * 128],
                        rhs=w_v_sb[kc],
                        start=(kc == 0),
                        stop=(kc == KC - 1),
                    )
                nc.vector.tensor_add(v_sb[lt][:, :, 0:DK], ps, vb_sb[:, :, 0:DK])

            def emit_outproj_unit(lt):
                yp = psum.tile([128, C], F32, tag="mm", bufs=2, name="psy")
                for pr in range(2):
                    nc.tensor.matmul(
                        yp,
                        lhsT=otp_sb[pr][:, lt * 128 : (lt + 1) * 128],
                        rhs=w_out_sb[:, pr, :],
                        start=(pr == 0),
                        stop=(pr == 1),
                    )
                ysb = work.tile([128, C], F16, tag="ysb", bufs=3, name="ysb")
                nc.vector.tensor_copy(ysb, yp)
                eng = nc.sync if lt % 2 == 0 else nc.scalar
                eng.dma_start(out=y[lt * 128 : (lt + 1) * 128, :], in_=ysb)

            def proj_units(lc, with_v=True):
                u = [(emit_qk_unit, (uu, lc)) for uu in range(HPC)]
                if with_v:
                    u += [(emit_v_unit, (lt,)) for lt in range(4 * lc, 4 * lc + 4)]
                return u

            def emit_attention(qb, feed_early, feed_late):
                nkj = 4 * qb + 4
                npairs = nkj // 2

                def st_exp(h, p):
                    st2 = psum.tile(
                        [128, 1024], F32, tag="st2", bufs=2, name="psst"
                    )
                    for half in range(2):
                        kj = 2 * p + half
                        r = kj - 4 * qb
                        # straddle tiles only feed queries at or beyond
                        # the diagonal: columns [128*r, 512)
                        ws = 128 * r if r > 0 else 0
                        nc.tensor.matmul(
                            st2[:, 512 * half + ws : 512 * (half + 1)],
                            lhsT=kT_sb[h][:, kj * 128 : (kj + 1) * 128],
                            rhs=qT_sb[h][:, qb * 512 + ws : (qb + 1) * 512],
                            start=True,
                            stop=True,
                        )
                    se = work.tile([128, 1024], mm, tag="se", bufs=4, name="se")
                    r0 = 2 * p - 4 * qb
                    if r0 >= 0:
                        # straddle pair: the two halves were written
                        # right-aligned with different starts; exp each half
                        # separately so only initialized PSUM is read.
                        for half in range(2):
                            ws = 128 * (r0 + half)
                            nc.scalar.activation(
                                out=se[:, 512 * half + ws : 512 * (half + 1)],
                                in_=st2[:, 512 * half + ws : 512 * (half + 1)],
                                func=mybir.ActivationFunctionType.Exp,
                                scale=float(SCALE),
                            )
                    else:
                        nc.scalar.activation(
                            out=se[:, 0:1024],
                            in_=st2[:, 0:1024],
                            func=mybir.ActivationFunctionType.Exp,
                            scale=float(SCALE),
                        )
                    return se

                def mask_ot(h, p, se, ot):
                    r0 = 2 * p - 4 * qb
                    if r0 >= 0:
                        # straddle pair: both halves carry a diagonal
                        # 128x128 block needing the triangle mask. The two
                        # blocks sit 640 columns apart in se -- one 3-dim
                        # DVE multiply covers both.
                        ws = 128 * r0 if r0 > 0 else 0
                        base = se[:, ws : ws + 128]
                        se2 = bass.AP(
                            tensor=base.tensor,
                            offset=base.offset,
                            ap=[base.ap[0], [640, 2], [1, 128]],
                        )
                        tri_ap = tri_sb[:, :]
                        tri2 = bass.AP(
                            tensor=tri_ap.tensor,
                            offset=tri_ap.offset,
                            ap=[tri_ap.ap[0], [0, 2], [1, 128]],
                        )
                        nc.vector.tensor_mul(se2, se2, tri2)
                    for half in range(2):
                        kj = 2 * p + half
                        r = kj - 4 * qb
                        ws = 128 * r if r > 0 else 0
                        o = 512 * half
                        nc.tensor.matmul(
                            ot[:, ws:512],
                            lhsT=v_sb[kj][:, h, :],
                            rhs=se[:, o + ws : o + 512],
                            start=(kj == 0),
                            stop=(kj == nkj - 1),
                        )

                def epilogue_a(h, ot):
                    # normalize: ot[:DK] /= ot[DK] -- all off the PE queue:
                    # DVE copy + fast reciprocal, GpSimd partition
                    # broadcast. (The copy must NOT go to ACT: it would
                    # queue ahead of upcoming exps in the ACT FIFO and
                    # stall the OTs.)
                    dnm = work.tile([1, 512], F32, tag="dnm", bufs=2, name="dnm")
                    nc.vector.tensor_copy(dnm, ot[DK : DK + 1, :])
                    rcp = work.tile([1, 512], F32, tag="rcp", bufs=2, name="rcp")
                    nc.vector.reciprocal_approx_fast(out=rcp, in_=dnm)
                    rbs = work.tile([DK, 512], F32, tag="rbs", bufs=2, name="rbs")
                    nc.gpsimd.partition_broadcast(rbs, rcp)
                    return (h, ot, rbs)

                def epilogue_b(h, ot, rbs):
                    # the final multiply is emitted ~2 slots after the
                    # broadcast was issued: if emitted immediately it
                    # head-of-line-blocks the in-order DVE queue waiting on
                    # GpSimd, delaying the projection-feed copies behind it
                    # (whose PSUM bufs the PE then stalls on).
                    rb = 64 * (h % 2)
                    nc.vector.tensor_mul(
                        otp_sb[h // 2][rb : rb + 64, qb * 512 : (qb + 1) * 512],
                        ot[0:DK, :],
                        rbs,
                    )

                # Per-head software pipeline, depth 2: ST/exp leads OT by
                # two pairs within each head. feed_early: one unit per slot
                # until exhausted (data needed soon). feed_late: head-start
                # slots first (the first OT of a head is most exposed to
                # exp latency), remainder spread evenly -- a pair is
                # ACT-heavier (~1.1us exp) than PE-heavy (~0.9us), so
                # clustering feed early would starve the PE at block end.
                nslots = HPC * npairs
                ne = (len(feed_early) + 1) // 2  # early units go 2 per slot
                nl = len(feed_late)
                head_starts = [h * npairs for h in range(HPC) if h * npairs >= ne]
                assigned = set(head_starts[:nl])
                rest = [s for s in range(ne, nslots) if s not in assigned]
                nrem = nl - len(assigned)
                if nrem > 0 and rest:
                    step = len(rest) / nrem
                    for i in range(nrem):
                        assigned.add(rest[min(int(i * step), len(rest) - 1)])
                for h in range(HPC):
                    ot = psum.tile([128, 512], F32, tag="ot", bufs=2, name="psot")
                    se_buf = {}
                    for p in range(min(2, npairs)):
                        se_buf[p] = st_exp(h, p)
                    for p in range(npairs):
                        s = h * npairs + p
                        if feed_early:
                            # up to two per slot: an OT pair consumes two v
                            # tiles, so the early v units must stay ahead
                            for _ in range(2):
                                if feed_early:
                                    fn, args = feed_early.pop(0)
                                    fn(*args)
                        elif s in assigned and feed_late:
                            fn, args = feed_late.pop(0)
                            fn(*args)
                        mask_ot(h, p, se_buf.pop(p), ot)
                        if p + 2 < npairs:
                            se_buf[p + 2] = st_exp(h, p + 2)
                    epilogue_b(*epilogue_a(h, ot))

            # prelude: slice-0 projections, then attention blocks. Feed
            # distribution tracks the PE-vs-ACT balance: early blocks have
            # plenty of projection feed; the final block (32 pair slots, no
            # projections left) gets slice-3's v units early (needed by its
            # own pair 6) plus ALL out-projections for rows 0..11 --
            # without them the PE would run ACT-bound there. Rows 12..15
            # drain at the tail.
            for fn, args in proj_units(0, with_v=False):
                fn(*args)
            for qb in range(QB):
                early, late = [], []
                if qb == 0:
                    # slice-0 v units ride qb0's early feed (2 per slot,
                    # staying ahead of the OT pairs) instead of extending
                    # the serial prelude behind the input-DMA wait
                    early += [(emit_v_unit, (lt,)) for lt in range(0, 4)]
                if qb + 1 < QB:
                    late += proj_units(qb + 1, with_v=(qb + 1 < 3))
                if qb == QB - 1:
                    early += [(emit_v_unit, (lt,)) for lt in range(12, 16)]
                    late += [(emit_outproj_unit, (lt,)) for lt in range(0, 12)]
                emit_attention(qb, early, late)
                for fn, args in early + late:
                    fn(*args)
                early, late = [], []
            # hold the clock-gate warm while the last head's epilogue
            # drains, so the final out-projection runs at 2.4 GHz
            for _ in range(5):
                nc.tensor.matmul(
                    wps, lhsT=warm[:, 0:128], rhs=warm, start=True, stop=True
                )
            for lt in range(12, LT):
                emit_outproj_unit(lt)

    nc.finalize()
    return nc


def _get_nc():
    if MM_MODE not in _CACHE:
        _CACHE[MM_MODE] = _build(MM_MODE)
    return _CACHE[MM_MODE]


def _make_tri():
    # [j, i] = 1 iff i >= j (key j attends-allowed for query i)
    return np.triu(np.ones((128, 128), np.float32))


def kernel(x, W_in, b_in, W_out, b_out):
    x = np.asarray(x, np.float32)
    W_in = np.asarray(W_in, np.float32)
    b_in = np.asarray(b_in, np.float32)
    W_out = np.asarray(W_out, np.float32)
    b_out = np.asarray(b_out, np.float32)

    mmd = _np_mm_dtype()
    tri = _make_tri().astype(mmd)

    in_maps = []
    for c in range(N_CORES):
        b, j = divmod(c, 2)
        w_in_loc = W_in[:, j * 768 : (j + 1) * 768]  # [C, 768]
        b_in_loc = b_in[j * 768 : (j + 1) * 768]  # [768]
        xT = np.ascontiguousarray(x[b].T).astype(mmd)  # [C, L]
        # pack qk columns into M=128 two-head units (see UNIT_DST in _build):
        # unit u halves: (low head = u%2==..) -> [role_h+2 | role_h] with
        # role q for even u, k for odd u, h = u//2
        wq = lambda h: w_in_loc[:, 192 * h : 192 * h + 64]
        wk = lambda h: w_in_loc[:, 192 * h + 64 : 192 * h + 128]
        bq = lambda h: b_in_loc[192 * h : 192 * h + 64]
        bk = lambda h: b_in_loc[192 * h + 64 : 192 * h + 128]
        units = [
            (wq(2), wq(0), bq(2), bq(0)),
            (wk(2), wk(0), bk(2), bk(0)),
            (wq(3), wq(1), bq(3), bq(1)),
            (wk(3), wk(1), bk(3), bk(1)),
        ]
        # w_in_qk: unit-major, partition-major rows ([u, p, kc, d]) so one
        # contiguous DMA delivers a whole unit's weights
        w_in_qk = np.zeros((HPC, 128, KC, 128), np.float32)
        w_in_v = np.zeros((C, HPC, DK), np.float32)
        qkb = np.zeros((128, 10), np.float32)
        for u, (wlo, whi, blo, bhi) in enumerate(units):
            wu = np.concatenate([wlo, whi], axis=1)  # [C, 128]
            w_in_qk[u] = wu.reshape(KC, 128, 128).transpose(1, 0, 2)
            qkb[0:64, 2 * u] = blo
            qkb[64:128, 2 * u + 1] = bhi
        qkb[0:64, 8] = 1.0  # low-half row mask
        qkb[64:128, 9] = 1.0  # high-half row mask
        for h in range(HPC):
            w_in_v[:, h, :] = w_in_loc[:, 192 * h + 128 : 192 * h + 192]
        w_in_qk = np.ascontiguousarray(w_in_qk).astype(mmd)
        w_in_v = np.ascontiguousarray(w_in_v).astype(mmd)
        vb = np.zeros((HPC, DK + 1), np.float32)
        for h in range(HPC):
            vb[h, :DK] = b_in_loc[192 * h + 128 : 192 * h + 192]
        # out-projection weights, head-PAIR packed to match otp_sb: row p of
        # pair pr is W_out row (head 2pr + p//64, dk p%64) of this core's
        # head group.
        w_out_loc = np.empty((128, 2, C), np.float32)
        for pr in range(2):
            for p in range(128):
                hh = j * HPC + 2 * pr + p // 64
                w_out_loc[p, pr, :] = W_out[hh * DK + p % 64, :]
        in_maps.append(
            dict(
                xT=xT,
                w_in_qk=w_in_qk,
                w_in_v=w_in_v,
                qkb=qkb,
                vb=vb,
                w_out=w_out_loc.astype(mmd),
                tri=tri,
            )
        )

    nc = _get_nc()
    res = run_bass_kernel_spmd(
        nc, in_maps, core_ids=list(range(N_CORES)), trace=TRACE
    )
    global LAST_RESULT
    LAST_RESULT = res

    out = np.empty((B, L, C), np.float32)
    for b in range(B):
        out[b] = (
            res.results[2 * b]["y"]
            + res.results[2 * b + 1]["y"]
            + b_out[None, :]
            + x[b]
        )
    return out
